# revision 10
# baseline (speedup 1.0000x reference)
"""Distributed causal multi-head attention for 8 TRN2 NeuronCores.

Problem: B=2, T=2048, D=1024, H=16 heads (hd=64), f32 in/out.

Sharding: core i handles batch b=i//4 and head-group g=i%4 (4 heads).
Wq/Wk/Wv column-sharded ([1024, 256] per core), Wo row-sharded
([256, 1024] per core).  Each core computes a partial output projection
for its 4 heads over the full sequence; the host sums the 4 partials
per batch (the unshard step replaces the all-reduce).  The host
pre-casts to bf16 and pre-blocks weights/activations into the on-chip
layout so every input tensor loads with a single large DMA descriptor.

Per-core dataflow (bf16 matmuls, f32 accumulation):
  QT,KT [128(2 heads x hd),T]  per m-group; heads 2m / 2m+1 at
  partitions 0:64 / 64:128.
  Scores for a head DUO (2m, 2m+1) run as row-tiled matmul pairs
  (K=64, tile_position rows 0/64) that execute CONCURRENTLY on the
  PE array, writing separate PSUM banks; one exp (ACT) covers both.
  V is augmented with a ones column, so AV [65, 512] carries the
  softmax denominator in row 64.  Normalization is transpose-free:
  reciprocal of the denominator row (DVE), gpsimd partition_broadcast,
  then one elementwise multiply writing attnT [d, q] directly (the
  DVE applies a partition-base shift for the odd head).
  Out-projection consumes attnT per t-tile; output is written bf16
  and summed in f32 on the host.

Pipeline: scores of duo p+2 are emitted during iteration p (pt pool
is triple-buffered), giving the ACT engine a two-iteration window per
duo so exp never gates the AV stream.  QKV projection work is woven
in as PE filler ordered by first use.
"""

import numpy as np
import ml_dtypes

import concourse.bass as bass
import concourse.mybir as mybir
import concourse.tile as tile
from concourse import bacc
from concourse.bass_utils import run_bass_kernel_spmd

F32 = mybir.dt.float32
BF16 = mybir.dt.bfloat16
AF = mybir.ActivationFunctionType

T = 2048  # sequence length
D = 1024  # embed dim
NH = 4  # heads per core
HD = 64  # head dim
DH = NH * HD  # 256, sharded d per core
TT = T // 128  # 16 t tiles
DT = D // 128  # 8 embed tiles
NSLAB = 4  # q slabs of 512
SCALE = 1.0 / np.sqrt(HD)

PT_MAX = 12 * 1024 + 2560  # pt cols for s=3

_NC_CACHE = None


def pt_layout(s):
    """Per-duo pt layout: (aoff, boff, width) per kt, and total cols.

    Off-diag kt < 4s: block of 1024 at 1024*kt (A at +0, B at +512).
    Diagonal j=kt-4s: widths 512-128j packed after, A then B.
    """
    aoff, boff, wid = {}, {}, {}
    for kt in range(4 * s):
        aoff[kt], boff[kt], wid[kt] = 1024 * kt, 1024 * kt + 512, 512
    base = 4096 * s
    wid[4 * s], wid[4 * s + 1], wid[4 * s + 2], wid[4 * s + 3] = 512, 384, 256, 128
    aoff[4 * s], boff[4 * s] = base, base + 512
    aoff[4 * s + 1], boff[4 * s + 1] = base + 1024, base + 1408
    # j2+j3 packed A-then-B so the concurrent A/B matmuls hit
    # different PSUM banks: A j2|j3 contiguous, then B j2|j3
    aoff[4 * s + 2], boff[4 * s + 2] = base + 1792, base + 2176
    aoff[4 * s + 3], boff[4 * s + 3] = base + 2048, base + 2432
    return aoff, boff, wid


def build():
    nc = bacc.Bacc(None, target_bir_lowering=False, debug=False)

    xT_ext = nc.declare_dram_parameter("xT", [128, DT * T], BF16, isOutput=False)
    wq = nc.declare_dram_parameter("Wq", [128, DT * DH], BF16, isOutput=False)
    wk = nc.declare_dram_parameter("Wk", [128, DT * DH], BF16, isOutput=False)
    wv = nc.declare_dram_parameter("Wv", [128, DT * DH], BF16, isOutput=False)
    wo = nc.declare_dram_parameter("Wo", [128, 2 * D], BF16, isOutput=False)
    out = nc.declare_dram_parameter("out", [T, D], BF16, isOutput=True)

    with tile.TileContext(nc) as tc:
        with (
            tc.tile_pool(name="persist", bufs=1) as persist,
            tc.tile_pool(name="pt", bufs=3) as pt_pool,
            tc.tile_pool(name="dnr", bufs=4) as dnr_pool,
            tc.tile_pool(name="dnb", bufs=4) as dnb_pool,
            tc.tile_pool(name="rcb", bufs=4) as rcb_pool,
            tc.tile_pool(name="ev", bufs=2) as ev_pool,
            tc.tile_pool(name="ps_sc", bufs=2, space="PSUM") as ps_sc,
            tc.tile_pool(name="ps_av", bufs=2, space="PSUM") as ps_av,
        ):
            def P(shape, dtype, name):
                return persist.tile(shape, dtype, name=name, tag=name)

            wq_bf = P([128, DT * DH], BF16, "wq_bf")
            wk_bf = P([128, DT * DH], BF16, "wk_bf")
            wv_bf = P([128, DT * DH], BF16, "wv_bf")
            wo_bf = P([128, 2 * D], BF16, "wo_bf")
            xT = P([128, DT * T], BF16, "xT")
            QT = P([128, 2 * T], BF16, "QT")
            KT = P([128, 2 * T], BF16, "KT")
            vbuf = P([128, TT * NH * 65], BF16, "vbuf")
            attnT = P([128, 2 * T], BF16, "attnT")

            # ---- input DMAs: one descriptor per tensor (plus an early
            # xT slice for the first projection chunk) ----
            xT3 = xT.rearrange("p (dt t) -> p dt t", t=T)
            xT3e = xT_ext.rearrange("p (dt t) -> p dt t", t=T)
            nc.scalar.dma_start(out=wq_bf[:], in_=wq[:])
            nc.sync.dma_start(out=xT3[:, :, 0:1024], in_=xT3e[:, :, 0:1024])
            nc.scalar.dma_start(out=wk_bf[:], in_=wk[:])
            nc.sync.dma_start(out=xT3[:, :, 1024:T], in_=xT3e[:, :, 1024:T])
            nc.scalar.dma_start(out=wv_bf[:], in_=wv[:])
            nc.scalar.dma_start(out=wo_bf[:], in_=wo[:])

            vb4 = vbuf.rearrange("p (n c) -> p n c", c=65)
            nc.gpsimd.memset(vb4[:, :, 64:65], 1.0)

            # ---- emitters ----
            def qk_chunk(w_bf, outT, m, c2):
                """One QK projection chunk: 1024 t-cols of head-duo m."""
                def go():
                    ps = ps_sc.tile([128, 1024], F32, name="psc")
                    for dt in range(DT):
                        lhsT = w_bf[:, dt * DH + m * 128 : dt * DH + (m + 1) * 128]
                        for half in range(2):
                            c0 = c2 * 1024 + half * 512
                            nc.tensor.matmul(
                                ps[:, half * 512 : (half + 1) * 512],
                                lhsT=lhsT,
                                rhs=xT[:, dt * T + c0 : dt * T + c0 + 512],
                                start=(dt == 0),
                                stop=(dt == DT - 1),
                            )
                    nc.vector.tensor_copy(
                        outT[:, m * T + c2 * 1024 : m * T + (c2 + 1) * 1024], ps[:]
                    )

                return go

            def v_quad(q):
                """V projection for t-tiles 4q..4q+3."""
                def go():
                    ps = ps_sc.tile([128, 1024], F32, name="psc")
                    for ti in range(4):
                        tt = 4 * q + ti
                        for dt in range(DT):
                            nc.tensor.matmul(
                                ps[:, ti * 256 : (ti + 1) * 256],
                                lhsT=xT[:, dt * T + tt * 128 : dt * T + (tt + 1) * 128],
                                rhs=wv_bf[:, dt * DH : (dt + 1) * DH],
                                start=(dt == 0),
                                stop=(dt == DT - 1),
                            )
                    nc.vector.tensor_copy(
                        vb4[:, 4 * q * NH : 4 * (q + 1) * NH, 0:64],
                        ps.rearrange("p (t c) -> p t c", c=64),
                    )

                return go

            def mask_diag(pt, col):
                """Causal triangle on the leading 128 cols of a diag block."""
                nc.gpsimd.affine_select(
                    out=pt[:, col : col + 128],
                    in_=pt[:, col : col + 128],
                    pattern=[[1, 128]],
                    compare_op=mybir.AluOpType.is_ge,
                    fill=0.0,
                    base=0,
                    channel_multiplier=-1,
                )

            def scores_ops(s, m, pt):
                """Row-tiled concurrent score pairs + exp for duo (s, m)."""
                aoff, boff, wid = pt_layout(s)
                q0 = m * T + s * 512

                def off_diag(kt):
                    def go():
                        ps = ps_sc.tile([128, 1024], F32, name="psc")
                        for i, r0 in enumerate((0, 64)):
                            nc.tensor.matmul(
                                ps[:, i * 512 : (i + 1) * 512],
                                lhsT=KT[
                                    r0 : r0 + 64,
                                    m * T + kt * 128 : m * T + (kt + 1) * 128,
                                ],
                                rhs=QT[r0 : r0 + 64, q0 : q0 + 512],
                                start=True,
                                stop=True,
                            )
                        nc.scalar.activation(
                            out=pt[:, aoff[kt] : aoff[kt] + 1024],
                            in_=ps[:],
                            func=AF.Exp,
                            scale=float(SCALE),
                        )

                    return go

                def diag0():
                    kt = 4 * s

                    def go():
                        ps = ps_sc.tile([128, 1024], F32, name="psc")
                        for i, r0 in enumerate((0, 64)):
                            nc.tensor.matmul(
                                ps[:, i * 512 : (i + 1) * 512],
                                lhsT=KT[
                                    r0 : r0 + 64,
                                    m * T + kt * 128 : m * T + (kt + 1) * 128,
                                ],
                                rhs=QT[r0 : r0 + 64, q0 : q0 + 512],
                                start=True,
                                stop=True,
                            )
                        nc.scalar.activation(
                            out=pt[:, aoff[kt] : aoff[kt] + 1024],
                            in_=ps[:],
                            func=AF.Exp,
                            scale=float(SCALE),
                        )
                        mask_diag(pt, aoff[kt])
                        mask_diag(pt, boff[kt])

                    return go

                def diag1():
                    kt = 4 * s + 1

                    def go():
                        ps = ps_sc.tile([128, 1024], F32, name="psc")
                        for i, r0 in enumerate((0, 64)):
                            nc.tensor.matmul(
                                ps[:, i * 512 : i * 512 + 384],
                                lhsT=KT[
                                    r0 : r0 + 64,
                                    m * T + kt * 128 : m * T + (kt + 1) * 128,
                                ],
                                rhs=QT[r0 : r0 + 64, q0 + 128 : q0 + 512],
                                start=True,
                                stop=True,
                            )
                        nc.scalar.activation(
                            out=pt[:, aoff[kt] : aoff[kt] + 384],
                            in_=ps[:, 0:384],
                            func=AF.Exp,
                            scale=float(SCALE),
                        )
                        nc.scalar.activation(
                            out=pt[:, boff[kt] : boff[kt] + 384],
                            in_=ps[:, 512:896],
                            func=AF.Exp,
                            scale=float(SCALE),
                        )
                        mask_diag(pt, aoff[kt])
                        mask_diag(pt, boff[kt])

                    return go

                def diag23():
                    kt2, kt3 = 4 * s + 2, 4 * s + 3

                    def go():
                        ps = ps_sc.tile([128, 1024], F32, name="psc")
                        # A tiles -> bank 0 (cols 0:384), B tiles -> bank 1
                        # (cols 512:896) so concurrent pairs never share a
                        # bank.
                        for i, r0 in enumerate((0, 64)):
                            nc.tensor.matmul(
                                ps[:, i * 512 : i * 512 + 256],
                                lhsT=KT[
                                    r0 : r0 + 64,
                                    m * T + kt2 * 128 : m * T + (kt2 + 1) * 128,
                                ],
                                rhs=QT[r0 : r0 + 64, q0 + 256 : q0 + 512],
                                start=True,
                                stop=True,
                            )
                        for i, r0 in enumerate((0, 64)):
                            nc.tensor.matmul(
                                ps[:, i * 512 + 256 : i * 512 + 384],
                                lhsT=KT[
                                    r0 : r0 + 64,
                                    m * T + kt3 * 128 : m * T + (kt3 + 1) * 128,
                                ],
                                rhs=QT[r0 : r0 + 64, q0 + 384 : q0 + 512],
                                start=True,
                                stop=True,
                            )
                        nc.scalar.activation(
                            out=pt[:, aoff[kt2] : aoff[kt2] + 384],
                            in_=ps[:, 0:384],
                            func=AF.Exp,
                            scale=float(SCALE),
                        )
                        nc.scalar.activation(
                            out=pt[:, boff[kt2] : boff[kt2] + 384],
                            in_=ps[:, 512:896],
                            func=AF.Exp,
                            scale=float(SCALE),
                        )
                        mask_diag(pt, aoff[kt2])
                        mask_diag(pt, boff[kt2])
                        mask_diag(pt, aoff[kt3])
                        mask_diag(pt, boff[kt3])

                    return go

                return (
                    [off_diag(kt) for kt in range(4 * s)]
                    + [diag0(), diag1(), diag23()]
                )

            def av_norm_ops(s, m, pt, split_last=False):
                """AV accumulation + transpose-free normalization.

                Returns ops: for each head i (A=even at attnT parts 0:64,
                B=odd at 64:128): AV matmul chain over kts, then
                reciprocal of the denominator row, partition_broadcast,
                and the normalizing multiply into attnT.
                """
                aoff, boff, wid = pt_layout(s)
                nk = 4 * (s + 1)
                stg = {}

                def av(i):
                    h = 2 * m + i
                    off_of = aoff if i == 0 else boff

                    def go():
                        if i == 0:
                            stg["avb"] = ps_av.tile(
                                [128, 1024], F32, name="psav", tag="psav"
                            )
                        avb = stg["avb"]
                        o0 = i * 512
                        for kt in range(nk):
                            off = max(0, 128 * (kt - 4 * s))
                            w = 512 - off
                            nc.tensor.matmul(
                                avb[0:65, o0 + off : o0 + 512],
                                lhsT=vb4[:, kt * NH + h, :],
                                rhs=pt[:, off_of[kt] : off_of[kt] + w],
                                start=(kt == 0),
                                stop=(kt == nk - 1),
                            )

                    return go

                def norm(i):
                    def go():
                        avb = stg["avb"]
                        o0 = i * 512
                        dnr = dnr_pool.tile([1, 512], F32, name="dnr")
                        nc.scalar.copy(dnr[:], avb[64:65, o0 : o0 + 512])
                        dnb = dnb_pool.tile([64, 512], F32, name="dnb")
                        nc.gpsimd.partition_broadcast(dnb[:], dnr[:])
                        rcb = rcb_pool.tile([64, 512], F32, name="rcb")
                        nc.vector.reciprocal(rcb[:], dnb[:])
                        nc.vector.tensor_tensor(
                            out=attnT[
                                i * 64 : (i + 1) * 64,
                                m * T + s * 512 : m * T + (s + 1) * 512,
                            ],
                            in0=avb[0:64, o0 : o0 + 512],
                            in1=rcb[:],
                            op=mybir.AluOpType.mult,
                        )

                    return go

                return [av(0), norm(0), av(1), norm(1)]

            def epi_ops(s):
                """Out-projection for slab s (4 t-tiles)."""
                ops = []
                for tt in range(4 * s, 4 * (s + 1)):
                    def go(tt=tt):
                        ps = ps_sc.tile([128, 1024], F32, name="psc")
                        for i in range(2):
                            lhsT = attnT[:, i * T + tt * 128 : i * T + (tt + 1) * 128]
                            for ec in range(2):
                                nc.tensor.matmul(
                                    ps[:, ec * 512 : (ec + 1) * 512],
                                    lhsT=lhsT,
                                    rhs=wo_bf[
                                        :, i * D + ec * 512 : i * D + (ec + 1) * 512
                                    ],
                                    start=(i == 0),
                                    stop=(i == 1),
                                )
                        ev = ev_pool.tile([128, 1024], BF16, name="ev")
                        nc.vector.tensor_copy(ev[:], ps[:])
                        eng = nc.sync if tt % 2 == 0 else nc.scalar
                        eng.dma_start(
                            out=out[tt * 128 : (tt + 1) * 128, :], in_=ev[:]
                        )

                    ops.append(go)
                return ops

            def interleave(a, b):
                if not a:
                    return list(b)
                if not b:
                    return list(a)
                res = []
                nb, na, bi = len(b), len(a), 0
                for i, op in enumerate(a):
                    res.append(op)
                    want = (i + 1) * nb // na
                    while bi < want:
                        res.append(b[bi])
                        bi += 1
                res.extend(b[bi:])
                return res

            # ---- pipeline ----
            duos = [(s, m) for s in range(NSLAB) for m in range(2)]
            pts = {}

            def new_pt(idx):
                pts[idx] = pt_pool.tile([128, PT_MAX], BF16, name="pt")
                return pts[idx]

            # prologue: projections for slab-0 scores of both duos; V
            # deferred past the first scores so exp starts early
            qk_chunk(wq_bf, QT, 0, 0)()
            qk_chunk(wk_bf, KT, 0, 0)()
            for op in scores_ops(0, 0, new_pt(0)):
                op()
            qk_chunk(wq_bf, QT, 1, 0)()
            qk_chunk(wk_bf, KT, 1, 0)()
            for op in scores_ops(0, 1, new_pt(1)):
                op()
            v_quad(0)()

            fillers = {
                0: [v_quad(1)],
                1: [qk_chunk(wq_bf, QT, 0, 1), qk_chunk(wk_bf, KT, 0, 1)],
                2: [qk_chunk(wq_bf, QT, 1, 1), qk_chunk(wk_bf, KT, 1, 1)],
                3: [v_quad(2)],
                4: [v_quad(3)],
            }

            for idx in range(8):
                sc = []
                if idx + 2 < 8:
                    s2, m2 = duos[idx + 2]
                    sc = scores_ops(s2, m2, new_pt(idx + 2))
                s_, m_ = duos[idx]
                avn = av_norm_ops(s_, m_, pts[idx])
                fill = fillers.get(idx, [])
                epi = []
                if idx >= 2 and idx % 2 == 0:
                    epi = epi_ops((idx - 2) // 2)
                for op in interleave(sc, fill + avn + epi):
                    op()
            # drain: last slab epilogue
            for op in epi_ops(3):
                op()

    nc.compile()
    return nc


def _get_nc():
    global _NC_CACHE
    if _NC_CACHE is None:
        _NC_CACHE = build()
    return _NC_CACHE


def make_in_maps(x, Wq, Wk, Wv, Wo):
    bf = ml_dtypes.bfloat16
    x = np.asarray(x, dtype=np.float32)
    Wq_ = np.asarray(Wq, dtype=np.float32)
    Wk_ = np.asarray(Wk, dtype=np.float32)
    Wv_ = np.asarray(Wv, dtype=np.float32)
    Wo_ = np.asarray(Wo, dtype=np.float32)

    def blk(a, p=128):
        # [n*p, c] -> [p, n*c] with n-blocks along columns
        n = a.shape[0] // p
        return np.ascontiguousarray(
            a.reshape(n, p, a.shape[1]).transpose(1, 0, 2).reshape(p, -1)
        )

    xTb = [blk(np.ascontiguousarray(x[b].T)).astype(bf) for b in range(2)]
    in_maps = []
    for core in range(8):
        b, g = core // 4, core % 4
        sl = slice(g * DH, (g + 1) * DH)
        in_maps.append(
            {
                "xT": xTb[b],
                "Wq": blk(Wq_[:, sl]).astype(bf),
                "Wk": blk(Wk_[:, sl]).astype(bf),
                "Wv": blk(Wv_[:, sl]).astype(bf),
                "Wo": blk(Wo_[sl, :]).astype(bf),
            }
        )
    return in_maps


def unshard(results):
    out = np.empty((2, T, D), np.float32)
    for b in range(2):
        out[b] = results[4 * b]["out"].astype(np.float32)
        for g in range(1, 4):
            out[b] += results[4 * b + g]["out"].astype(np.float32)
    return out


def kernel(x, Wq, Wk, Wv, Wo):
    nc = _get_nc()
    in_maps = make_in_maps(x, Wq, Wk, Wv, Wo)
    res = run_bass_kernel_spmd(nc, in_maps, core_ids=list(range(8)))
    return unshard(res.results)


# revision 21
# speedup vs baseline: 1.3217x; 1.3217x over previous
"""Distributed causal multi-head attention for 8 TRN2 NeuronCores.

Problem: B=2, T=2048, D=1024, H=16 heads (hd=64), f32 in/out.

Sharding: core i handles batch b=i//4 and head-group g=i%4 (4 heads).
Wq/Wk/Wv column-sharded ([1024, 256] per core), Wo row-sharded
([256, 1024] per core).  Each core computes a partial output projection
for its 4 heads over the full sequence; the host sums the 4 partials
per batch (the unshard step replaces the all-reduce).  The host
pre-casts to bf16 and pre-blocks weights/activations into the on-chip
layout so every input tensor loads with a single large DMA descriptor.

Per-core dataflow (bf16 matmuls, f32 accumulation):
  QT,KT [128(2 heads x hd),T]  per m-group; heads 2m / 2m+1 at
  partitions 0:64 / 64:128.
  Scores for a head DUO (2m, 2m+1) run as row-tiled matmul pairs
  (K=64, tile_position rows 0/64) that execute CONCURRENTLY on the
  PE array, writing separate PSUM banks; one exp (ACT) covers both.
  V is augmented with a ones column, so AV [65, 512] carries the
  softmax denominator in row 64.  Normalization is transpose-free:
  reciprocal of the denominator row (DVE), gpsimd partition_broadcast,
  then one elementwise multiply writing attnT [d, q] directly (the
  DVE applies a partition-base shift for the odd head).
  Out-projection consumes attnT per t-tile; output is written bf16
  and summed in f32 on the host.

Pipeline: scores of duo p+2 are emitted during iteration p (pt pool
is triple-buffered), giving the ACT engine a two-iteration window per
duo so exp never gates the AV stream.  QKV projection work is woven
in as PE filler ordered by first use.
"""

import numpy as np
import ml_dtypes

import concourse.bass as bass
import concourse.mybir as mybir
import concourse.tile as tile
from concourse import bacc
from concourse.bass_utils import run_bass_kernel_spmd

F32 = mybir.dt.float32
BF16 = mybir.dt.bfloat16
AF = mybir.ActivationFunctionType

T = 2048  # sequence length
D = 1024  # embed dim
NH = 4  # heads per core
HD = 64  # head dim
DH = NH * HD  # 256, sharded d per core
TT = T // 128  # 16 t tiles
DT = D // 128  # 8 embed tiles
NSLAB = 4  # q slabs of 512
SCALE = 1.0 / np.sqrt(HD)

PT_MAX = 12 * 1024 + 2560  # pt cols for s=3

_NC_CACHE = None


def pt_layout(s):
    """Per-duo pt layout: (aoff, boff, width) per kt, and total cols.

    Off-diag kt < 4s: block of 1024 at 1024*kt (A at +0, B at +512).
    Diagonal j=kt-4s: widths 512-128j packed after, A then B.
    """
    aoff, boff, wid = {}, {}, {}
    for kt in range(4 * s):
        aoff[kt], boff[kt], wid[kt] = 1024 * kt, 1024 * kt + 512, 512
    base = 4096 * s
    wid[4 * s], wid[4 * s + 1], wid[4 * s + 2], wid[4 * s + 3] = 512, 384, 256, 128
    aoff[4 * s], boff[4 * s] = base, base + 512
    aoff[4 * s + 1], boff[4 * s + 1] = base + 1024, base + 1408
    # j2+j3 packed A-then-B so the concurrent A/B matmuls hit
    # different PSUM banks: A j2|j3 contiguous, then B j2|j3
    aoff[4 * s + 2], boff[4 * s + 2] = base + 1792, base + 2176
    aoff[4 * s + 3], boff[4 * s + 3] = base + 2048, base + 2432
    return aoff, boff, wid


def build():
    nc = bacc.Bacc(None, target_bir_lowering=False, debug=False)

    xT_ext = nc.declare_dram_parameter("xT", [128, DT * T], BF16, isOutput=False)
    wq = nc.declare_dram_parameter("Wq", [128, DT * DH], BF16, isOutput=False)
    wk = nc.declare_dram_parameter("Wk", [128, DT * DH], BF16, isOutput=False)
    wv = nc.declare_dram_parameter("Wv", [128, DT * DH], BF16, isOutput=False)
    wo = nc.declare_dram_parameter("Wo", [128, 2 * D], BF16, isOutput=False)
    out = nc.declare_dram_parameter("out", [T, D], BF16, isOutput=True)

    with tile.TileContext(nc) as tc:
        with (
            tc.tile_pool(name="persist", bufs=1) as persist,
            tc.tile_pool(name="pt", bufs=3) as pt_pool,
            tc.tile_pool(name="rcb", bufs=4) as rcb_pool,
            tc.tile_pool(name="ev", bufs=2) as ev_pool,
            tc.tile_pool(name="ps_sc", bufs=2, space="PSUM") as ps_sc,
            tc.tile_pool(name="ps_av", bufs=2, space="PSUM") as ps_av,
        ):
            def P(shape, dtype, name):
                return persist.tile(shape, dtype, name=name, tag=name)

            wq_bf = P([128, DT * DH], BF16, "wq_bf")
            wk_bf = P([128, DT * DH], BF16, "wk_bf")
            wv_bf = P([128, DT * DH], BF16, "wv_bf")
            wo_bf = P([128, 2 * D], BF16, "wo_bf")
            xT = P([128, DT * T], BF16, "xT")
            QT = P([128, 2 * T], BF16, "QT")
            KT = P([128, 2 * T], BF16, "KT")
            vbuf = P([128, TT * NH * 128], BF16, "vbuf")
            attnT = P([128, 2 * T], BF16, "attnT")

            # ---- input DMAs: one descriptor per tensor (plus an early
            # xT slice for the first projection chunk) ----
            xT3 = xT.rearrange("p (dt t) -> p dt t", t=T)
            xT3e = xT_ext.rearrange("p (dt t) -> p dt t", t=T)
            nc.scalar.dma_start(out=wq_bf[:], in_=wq[:])
            nc.sync.dma_start(out=xT3[:, :, 0:512], in_=xT3e[:, :, 0:512])
            nc.scalar.dma_start(out=wk_bf[:], in_=wk[:])
            nc.sync.dma_start(out=xT3[:, :, 512:1024], in_=xT3e[:, :, 512:1024])
            nc.scalar.dma_start(out=wv_bf[:], in_=wv[:])
            nc.sync.dma_start(out=xT3[:, :, 1024:T], in_=xT3e[:, :, 1024:T])
            nc.scalar.dma_start(out=wo_bf[:], in_=wo[:])

            # cols 0:64 of every V block are ones: the AV matmul then
            # emits the softmax denominator replicated across PSUM rows
            # 0:64 (M is free - matmul time is N-bound), giving a
            # base-0 [64, 512] operand for the fast approx-reciprocal
            # (the custom DVE op only works at partition base 0).
            vb4 = vbuf.rearrange("p (n c) -> p n c", c=128)
            nc.gpsimd.memset(vb4[:, :, 0:64], 1.0)

            # ---- emitters ----
            def qk_chunk(w_bf, outT, m, c2):
                """One QK projection chunk: 1024 t-cols of head-duo m."""
                def go():
                    ps = ps_sc.tile([128, 1024], F32, name="psc")
                    for dt in range(DT):
                        lhsT = w_bf[:, dt * DH + m * 128 : dt * DH + (m + 1) * 128]
                        for half in range(2):
                            c0 = c2 * 1024 + half * 512
                            nc.tensor.matmul(
                                ps[:, half * 512 : (half + 1) * 512],
                                lhsT=lhsT,
                                rhs=xT[:, dt * T + c0 : dt * T + c0 + 512],
                                start=(dt == 0),
                                stop=(dt == DT - 1),
                            )
                    nc.vector.tensor_copy(
                        outT[:, m * T + c2 * 1024 : m * T + (c2 + 1) * 1024], ps[:]
                    )

                return go

            def v_quad(q):
                """V projection for t-tiles 4q..4q+3."""
                def go():
                    ps = ps_sc.tile([128, 1024], F32, name="psc")
                    for ti in range(4):
                        tt = 4 * q + ti
                        for dt in range(DT):
                            nc.tensor.matmul(
                                ps[:, ti * 256 : (ti + 1) * 256],
                                lhsT=xT[:, dt * T + tt * 128 : dt * T + (tt + 1) * 128],
                                rhs=wv_bf[:, dt * DH : (dt + 1) * DH],
                                start=(dt == 0),
                                stop=(dt == DT - 1),
                            )
                    nc.vector.tensor_copy(
                        vb4[:, 4 * q * NH : 4 * (q + 1) * NH, 64:128],
                        ps.rearrange("p (t c) -> p t c", c=64),
                    )

                return go

            def mask_diag(pt, col):
                """Causal triangle on the leading 128 cols of a diag block."""
                nc.gpsimd.affine_select(
                    out=pt[:, col : col + 128],
                    in_=pt[:, col : col + 128],
                    pattern=[[1, 128]],
                    compare_op=mybir.AluOpType.is_ge,
                    fill=0.0,
                    base=0,
                    channel_multiplier=-1,
                )

            def scores_ops(s, m, pt):
                """Row-tiled concurrent score pairs + exp for duo (s, m)."""
                aoff, boff, wid = pt_layout(s)
                q0 = m * T + s * 512

                def off_diag(kt):
                    def go():
                        ps = ps_sc.tile([128, 1024], F32, name="psc")
                        for i, r0 in enumerate((0, 64)):
                            nc.tensor.matmul(
                                ps[:, i * 512 : (i + 1) * 512],
                                lhsT=KT[
                                    r0 : r0 + 64,
                                    m * T + kt * 128 : m * T + (kt + 1) * 128,
                                ],
                                rhs=QT[r0 : r0 + 64, q0 : q0 + 512],
                                start=True,
                                stop=True,
                            )
                        nc.scalar.activation(
                            out=pt[:, aoff[kt] : aoff[kt] + 1024],
                            in_=ps[:],
                            func=AF.Exp,
                            scale=float(SCALE),
                        )

                    return go

                def diag0():
                    kt = 4 * s

                    def go():
                        ps = ps_sc.tile([128, 1024], F32, name="psc")
                        for i, r0 in enumerate((0, 64)):
                            nc.tensor.matmul(
                                ps[:, i * 512 : (i + 1) * 512],
                                lhsT=KT[
                                    r0 : r0 + 64,
                                    m * T + kt * 128 : m * T + (kt + 1) * 128,
                                ],
                                rhs=QT[r0 : r0 + 64, q0 : q0 + 512],
                                start=True,
                                stop=True,
                            )
                        nc.scalar.activation(
                            out=pt[:, aoff[kt] : aoff[kt] + 1024],
                            in_=ps[:],
                            func=AF.Exp,
                            scale=float(SCALE),
                        )
                        mask_diag(pt, aoff[kt])
                        mask_diag(pt, boff[kt])

                    return go

                def diag1():
                    kt = 4 * s + 1

                    def go():
                        ps = ps_sc.tile([128, 1024], F32, name="psc")
                        for i, r0 in enumerate((0, 64)):
                            nc.tensor.matmul(
                                ps[:, i * 512 : i * 512 + 384],
                                lhsT=KT[
                                    r0 : r0 + 64,
                                    m * T + kt * 128 : m * T + (kt + 1) * 128,
                                ],
                                rhs=QT[r0 : r0 + 64, q0 + 128 : q0 + 512],
                                start=True,
                                stop=True,
                            )
                        nc.scalar.activation(
                            out=pt[:, aoff[kt] : aoff[kt] + 384],
                            in_=ps[:, 0:384],
                            func=AF.Exp,
                            scale=float(SCALE),
                        )
                        nc.scalar.activation(
                            out=pt[:, boff[kt] : boff[kt] + 384],
                            in_=ps[:, 512:896],
                            func=AF.Exp,
                            scale=float(SCALE),
                        )
                        mask_diag(pt, aoff[kt])
                        mask_diag(pt, boff[kt])

                    return go

                def diag23():
                    kt2, kt3 = 4 * s + 2, 4 * s + 3

                    def go():
                        ps = ps_sc.tile([128, 1024], F32, name="psc")
                        # A tiles -> bank 0 (cols 0:384), B tiles -> bank 1
                        # (cols 512:896) so concurrent pairs never share a
                        # bank.
                        for i, r0 in enumerate((0, 64)):
                            nc.tensor.matmul(
                                ps[:, i * 512 : i * 512 + 256],
                                lhsT=KT[
                                    r0 : r0 + 64,
                                    m * T + kt2 * 128 : m * T + (kt2 + 1) * 128,
                                ],
                                rhs=QT[r0 : r0 + 64, q0 + 256 : q0 + 512],
                                start=True,
                                stop=True,
                            )
                        for i, r0 in enumerate((0, 64)):
                            nc.tensor.matmul(
                                ps[:, i * 512 + 256 : i * 512 + 384],
                                lhsT=KT[
                                    r0 : r0 + 64,
                                    m * T + kt3 * 128 : m * T + (kt3 + 1) * 128,
                                ],
                                rhs=QT[r0 : r0 + 64, q0 + 384 : q0 + 512],
                                start=True,
                                stop=True,
                            )
                        nc.scalar.activation(
                            out=pt[:, aoff[kt2] : aoff[kt2] + 384],
                            in_=ps[:, 0:384],
                            func=AF.Exp,
                            scale=float(SCALE),
                        )
                        nc.scalar.activation(
                            out=pt[:, boff[kt2] : boff[kt2] + 384],
                            in_=ps[:, 512:896],
                            func=AF.Exp,
                            scale=float(SCALE),
                        )
                        mask_diag(pt, aoff[kt2])
                        mask_diag(pt, boff[kt2])
                        mask_diag(pt, aoff[kt3])
                        mask_diag(pt, boff[kt3])

                    return go

                return (
                    [off_diag(kt) for kt in range(4 * s)]
                    + [diag0(), diag1(), diag23()]
                )

            def av_norm_ops(s, m, pt, split_last=False):
                """AV accumulation + transpose-free normalization.

                Returns ops: for each head i (A=even at attnT parts 0:64,
                B=odd at 64:128): AV matmul chain over kts, then
                reciprocal of the denominator row, partition_broadcast,
                and the normalizing multiply into attnT.
                """
                aoff, boff, wid = pt_layout(s)
                nk = 4 * (s + 1)
                stg = {}

                def av_group(i, kts):
                    h = 2 * m + i
                    off_of = aoff if i == 0 else boff

                    def go():
                        if i == 0 and kts[0] == 0:
                            stg["avb"] = ps_av.tile(
                                [128, 1024], F32, name="psav", tag="psav"
                            )
                        avb = stg["avb"]
                        o0 = i * 512
                        for kt in kts:
                            off = max(0, 128 * (kt - 4 * s))
                            w = 512 - off
                            nc.tensor.matmul(
                                avb[:, o0 + off : o0 + 512],
                                lhsT=vb4[:, kt * NH + h, :],
                                rhs=pt[:, off_of[kt] : off_of[kt] + w],
                                start=(kt == 0),
                                stop=(kt == nk - 1),
                            )

                    return go

                def norm(i):
                    def go():
                        avb = stg["avb"]
                        o0 = i * 512
                        rcb = rcb_pool.tile([64, 512], F32, name="rcb")
                        nc.vector.reciprocal_approx_fast(
                            out=rcb[:], in_=avb[0:64, o0 : o0 + 512]
                        )
                        nc.vector.tensor_tensor(
                            out=attnT[
                                i * 64 : (i + 1) * 64,
                                m * T + s * 512 : m * T + (s + 1) * 512,
                            ],
                            in0=avb[64:128, o0 : o0 + 512],
                            in1=rcb[:],
                            op=mybir.AluOpType.mult,
                        )

                    return go

                def groups(i):
                    ks = list(range(nk))
                    return [av_group(i, ks[j : j + 4]) for j in range(0, nk, 4)]

                return groups(0) + [norm(0)] + groups(1) + [norm(1)]

            def epi_ops(s):
                """Out-projection for slab s (4 t-tiles)."""
                ops = []
                for tt in range(4 * s, 4 * (s + 1)):
                    def go(tt=tt):
                        ps = ps_sc.tile([128, 1024], F32, name="psc")
                        for i in range(2):
                            lhsT = attnT[:, i * T + tt * 128 : i * T + (tt + 1) * 128]
                            for ec in range(2):
                                nc.tensor.matmul(
                                    ps[:, ec * 512 : (ec + 1) * 512],
                                    lhsT=lhsT,
                                    rhs=wo_bf[
                                        :, i * D + ec * 512 : i * D + (ec + 1) * 512
                                    ],
                                    start=(i == 0),
                                    stop=(i == 1),
                                )
                        ev = ev_pool.tile([128, 1024], BF16, name="ev")
                        nc.vector.tensor_copy(ev[:], ps[:])
                        eng = nc.sync if tt % 2 == 0 else nc.scalar
                        eng.dma_start(
                            out=out[tt * 128 : (tt + 1) * 128, :], in_=ev[:]
                        )

                    ops.append(go)
                return ops

            def interleave(a, b):
                if not a:
                    return list(b)
                if not b:
                    return list(a)
                res = []
                nb, na, bi = len(b), len(a), 0
                for i, op in enumerate(a):
                    res.append(op)
                    want = (i + 1) * nb // na
                    while bi < want:
                        res.append(b[bi])
                        bi += 1
                res.extend(b[bi:])
                return res

            # ---- pipeline ----
            duos = [(s, m) for s in range(NSLAB) for m in range(2)]
            pts = {}

            def new_pt(idx):
                pts[idx] = pt_pool.tile([128, PT_MAX], BF16, name="pt")
                return pts[idx]

            # prologue: projections for slab-0 scores of both duos; V
            # deferred past the first scores so exp starts early
            qk_chunk(wq_bf, QT, 0, 0)()
            qk_chunk(wk_bf, KT, 0, 0)()
            for op in scores_ops(0, 0, new_pt(0)):
                op()
            qk_chunk(wq_bf, QT, 1, 0)()
            qk_chunk(wk_bf, KT, 1, 0)()
            for op in scores_ops(0, 1, new_pt(1)):
                op()
            v_quad(0)()

            fillers = {
                0: [v_quad(1)],
                1: [qk_chunk(wq_bf, QT, 0, 1), qk_chunk(wk_bf, KT, 0, 1)],
                2: [qk_chunk(wq_bf, QT, 1, 1), qk_chunk(wk_bf, KT, 1, 1)],
                3: [v_quad(2)],
                4: [v_quad(3)],
            }

            for idx in range(8):
                sc = []
                if idx + 2 < 8:
                    s2, m2 = duos[idx + 2]
                    sc = scores_ops(s2, m2, new_pt(idx + 2))
                s_, m_ = duos[idx]
                avn = av_norm_ops(s_, m_, pts[idx])
                fill = fillers.get(idx, [])
                epi = []
                if idx >= 2 and idx % 2 == 0:
                    epi = epi_ops((idx - 2) // 2)
                for op in interleave(sc, fill + avn + epi):
                    op()
            # drain: last slab epilogue
            for op in epi_ops(3):
                op()

    nc.compile()
    return nc


def _get_nc():
    global _NC_CACHE
    if _NC_CACHE is None:
        _NC_CACHE = build()
    return _NC_CACHE


def make_in_maps(x, Wq, Wk, Wv, Wo):
    bf = ml_dtypes.bfloat16
    x = np.asarray(x, dtype=np.float32)
    Wq_ = np.asarray(Wq, dtype=np.float32)
    Wk_ = np.asarray(Wk, dtype=np.float32)
    Wv_ = np.asarray(Wv, dtype=np.float32)
    Wo_ = np.asarray(Wo, dtype=np.float32)

    def blk(a, p=128):
        # [n*p, c] -> [p, n*c] with n-blocks along columns
        n = a.shape[0] // p
        return np.ascontiguousarray(
            a.reshape(n, p, a.shape[1]).transpose(1, 0, 2).reshape(p, -1)
        )

    xTb = [blk(np.ascontiguousarray(x[b].T)).astype(bf) for b in range(2)]
    in_maps = []
    for core in range(8):
        b, g = core // 4, core % 4
        sl = slice(g * DH, (g + 1) * DH)
        in_maps.append(
            {
                "xT": xTb[b],
                "Wq": blk(Wq_[:, sl]).astype(bf),
                "Wk": blk(Wk_[:, sl]).astype(bf),
                "Wv": blk(Wv_[:, sl]).astype(bf),
                "Wo": blk(Wo_[sl, :]).astype(bf),
            }
        )
    return in_maps


def unshard(results):
    out = np.empty((2, T, D), np.float32)
    for b in range(2):
        out[b] = results[4 * b]["out"].astype(np.float32)
        for g in range(1, 4):
            out[b] += results[4 * b + g]["out"].astype(np.float32)
    return out


def kernel(x, Wq, Wk, Wv, Wo):
    nc = _get_nc()
    in_maps = make_in_maps(x, Wq, Wk, Wv, Wo)
    res = run_bass_kernel_spmd(nc, in_maps, core_ids=list(range(8)))
    return unshard(res.results)


# revision 24
# speedup vs baseline: 1.3335x; 1.0090x over previous
"""Distributed causal multi-head attention for 8 TRN2 NeuronCores.

Problem: B=2, T=2048, D=1024, H=16 heads (hd=64), f32 in/out.

Sharding: core i handles batch b=i//4 and head-group g=i%4 (4 heads).
Wq/Wk/Wv column-sharded ([1024, 256] per core), Wo row-sharded
([256, 1024] per core).  Each core computes a partial output projection
for its 4 heads over the full sequence; the host sums the 4 partials
per batch (the unshard step replaces the all-reduce).  The host
pre-casts to bf16 and pre-blocks weights/activations into the on-chip
layout so every input tensor loads with a single large DMA descriptor.

Per-core dataflow (bf16 matmuls, f32 accumulation):
  QT,KT [128(2 heads x hd),T]  per m-group; heads 2m / 2m+1 at
  partitions 0:64 / 64:128.
  Scores for a head DUO (2m, 2m+1) run as row-tiled matmul pairs
  (K=64, tile_position rows 0/64) that execute CONCURRENTLY on the
  PE array, writing separate PSUM banks; one exp (ACT) covers both.
  V is augmented with a ones column, so AV [65, 512] carries the
  softmax denominator in row 64.  Normalization is transpose-free:
  reciprocal of the denominator row (DVE), gpsimd partition_broadcast,
  then one elementwise multiply writing attnT [d, q] directly (the
  DVE applies a partition-base shift for the odd head).
  Out-projection consumes attnT per t-tile; output is written bf16
  and summed in f32 on the host.

Pipeline: scores of duo p+2 are emitted during iteration p (pt pool
is triple-buffered), giving the ACT engine a two-iteration window per
duo so exp never gates the AV stream.  QKV projection work is woven
in as PE filler ordered by first use.
"""

import numpy as np
import ml_dtypes

import concourse.bass as bass
import concourse.mybir as mybir
import concourse.tile as tile
from concourse import bacc
from concourse.bass_utils import run_bass_kernel_spmd

F32 = mybir.dt.float32
BF16 = mybir.dt.bfloat16
AF = mybir.ActivationFunctionType

T = 2048  # sequence length
D = 1024  # embed dim
NH = 4  # heads per core
HD = 64  # head dim
DH = NH * HD  # 256, sharded d per core
TT = T // 128  # 16 t tiles
DT = D // 128  # 8 embed tiles
NSLAB = 4  # q slabs of 512
SCALE = 1.0 / np.sqrt(HD)

PT_MAX = 12 * 1024 + 2560  # pt cols for s=3

_NC_CACHE = None


def pt_layout(s):
    """Per-duo pt layout: (aoff, boff, width) per kt, and total cols.

    Off-diag kt < 4s: block of 1024 at 1024*kt (A at +0, B at +512).
    Diagonal j=kt-4s: widths 512-128j packed after, A then B.
    """
    aoff, boff, wid = {}, {}, {}
    for kt in range(4 * s):
        aoff[kt], boff[kt], wid[kt] = 1024 * kt, 1024 * kt + 512, 512
    base = 4096 * s
    wid[4 * s], wid[4 * s + 1], wid[4 * s + 2], wid[4 * s + 3] = 512, 384, 256, 128
    aoff[4 * s], boff[4 * s] = base, base + 512
    aoff[4 * s + 1], boff[4 * s + 1] = base + 1024, base + 1408
    # j2+j3 packed A-then-B so the concurrent A/B matmuls hit
    # different PSUM banks: A j2|j3 contiguous, then B j2|j3
    aoff[4 * s + 2], boff[4 * s + 2] = base + 1792, base + 2176
    aoff[4 * s + 3], boff[4 * s + 3] = base + 2048, base + 2432
    return aoff, boff, wid


def build():
    nc = bacc.Bacc(None, target_bir_lowering=False, debug=False)

    xT_ext = nc.declare_dram_parameter("xT", [128, DT * T], BF16, isOutput=False)
    wq = nc.declare_dram_parameter("Wq", [128, DT * DH], BF16, isOutput=False)
    wk = nc.declare_dram_parameter("Wk", [128, DT * DH], BF16, isOutput=False)
    wv = nc.declare_dram_parameter("Wv", [128, DT * DH], BF16, isOutput=False)
    wo = nc.declare_dram_parameter("Wo", [128, 2 * D], BF16, isOutput=False)
    out = nc.declare_dram_parameter("out", [T, D], BF16, isOutput=True)

    with tile.TileContext(nc) as tc:
        with (
            tc.tile_pool(name="persist", bufs=1) as persist,
            tc.tile_pool(name="pt", bufs=3) as pt_pool,
            tc.tile_pool(name="rcb", bufs=4) as rcb_pool,
            tc.tile_pool(name="ev", bufs=2) as ev_pool,
            tc.tile_pool(name="ps_sc", bufs=2, space="PSUM") as ps_sc,
            tc.tile_pool(name="ps_av", bufs=2, space="PSUM") as ps_av,
        ):
            def P(shape, dtype, name):
                return persist.tile(shape, dtype, name=name, tag=name)

            wq_bf = P([128, DT * DH], BF16, "wq_bf")
            wk_bf = P([128, DT * DH], BF16, "wk_bf")
            wv_bf = P([128, DT * DH], BF16, "wv_bf")
            wo_bf = P([128, 2 * D], BF16, "wo_bf")
            xT = P([128, DT * T], BF16, "xT")
            QT = P([128, 2 * T], BF16, "QT")
            KT = P([128, 2 * T], BF16, "KT")
            vbuf = P([128, TT * NH * 128], BF16, "vbuf")
            attnT = P([128, 2 * T], BF16, "attnT")

            # ---- input DMAs: one descriptor per tensor (plus an early
            # xT slice for the first projection chunk) ----
            xT3 = xT.rearrange("p (dt t) -> p dt t", t=T)
            xT3e = xT_ext.rearrange("p (dt t) -> p dt t", t=T)
            nc.scalar.dma_start(out=wq_bf[:], in_=wq[:])
            nc.sync.dma_start(out=xT3[:, :, 0:512], in_=xT3e[:, :, 0:512])
            nc.scalar.dma_start(out=wk_bf[:], in_=wk[:])
            nc.sync.dma_start(out=xT3[:, :, 512:1024], in_=xT3e[:, :, 512:1024])
            nc.scalar.dma_start(out=wv_bf[:], in_=wv[:])
            nc.sync.dma_start(out=xT3[:, :, 1024:T], in_=xT3e[:, :, 1024:T])
            nc.scalar.dma_start(out=wo_bf[:], in_=wo[:])

            # cols 0:64 of every V block are ones: the AV matmul then
            # emits the softmax denominator replicated across PSUM rows
            # 0:64 (M is free - matmul time is N-bound), giving a
            # base-0 [64, 512] operand for the fast approx-reciprocal
            # (the custom DVE op only works at partition base 0).
            vb4 = vbuf.rearrange("p (n c) -> p n c", c=128)
            nc.gpsimd.memset(vb4[:, :, 0:64], 1.0)

            # ---- emitters ----
            def qk_chunk(w_bf, outT, m, c2):
                """One QK projection chunk: 1024 t-cols of head-duo m."""
                def go():
                    ps = ps_sc.tile([128, 1024], F32, name="psc")
                    # half-outer so the first 8 matmuls only need the first
                    # 512-col xT wave (startup-critical)
                    for half in range(2):
                        c0 = c2 * 1024 + half * 512
                        for dt in range(DT):
                            nc.tensor.matmul(
                                ps[:, half * 512 : (half + 1) * 512],
                                lhsT=w_bf[
                                    :, dt * DH + m * 128 : dt * DH + (m + 1) * 128
                                ],
                                rhs=xT[:, dt * T + c0 : dt * T + c0 + 512],
                                start=(dt == 0),
                                stop=(dt == DT - 1),
                            )
                    nc.vector.tensor_copy(
                        outT[:, m * T + c2 * 1024 : m * T + (c2 + 1) * 1024], ps[:]
                    )

                return go

            def v_quad(q):
                """V projection for t-tiles 4q..4q+3."""
                def go():
                    ps = ps_sc.tile([128, 1024], F32, name="psc")
                    for ti in range(4):
                        tt = 4 * q + ti
                        for dt in range(DT):
                            nc.tensor.matmul(
                                ps[:, ti * 256 : (ti + 1) * 256],
                                lhsT=xT[:, dt * T + tt * 128 : dt * T + (tt + 1) * 128],
                                rhs=wv_bf[:, dt * DH : (dt + 1) * DH],
                                start=(dt == 0),
                                stop=(dt == DT - 1),
                            )
                    nc.vector.tensor_copy(
                        vb4[:, 4 * q * NH : 4 * (q + 1) * NH, 64:128],
                        ps.rearrange("p (t c) -> p t c", c=64),
                    )

                return go

            def mask_diag(pt, col):
                """Causal triangle on the leading 128 cols of a diag block."""
                nc.gpsimd.affine_select(
                    out=pt[:, col : col + 128],
                    in_=pt[:, col : col + 128],
                    pattern=[[1, 128]],
                    compare_op=mybir.AluOpType.is_ge,
                    fill=0.0,
                    base=0,
                    channel_multiplier=-1,
                )

            def scores_ops(s, m, pt):
                """Row-tiled concurrent score pairs + exp for duo (s, m)."""
                aoff, boff, wid = pt_layout(s)
                q0 = m * T + s * 512

                def off_diag(kt):
                    def go():
                        ps = ps_sc.tile([128, 1024], F32, name="psc")
                        for i, r0 in enumerate((0, 64)):
                            nc.tensor.matmul(
                                ps[:, i * 512 : (i + 1) * 512],
                                lhsT=KT[
                                    r0 : r0 + 64,
                                    m * T + kt * 128 : m * T + (kt + 1) * 128,
                                ],
                                rhs=QT[r0 : r0 + 64, q0 : q0 + 512],
                                start=True,
                                stop=True,
                            )
                        nc.scalar.activation(
                            out=pt[:, aoff[kt] : aoff[kt] + 1024],
                            in_=ps[:],
                            func=AF.Exp,
                            scale=float(SCALE),
                        )

                    return go

                def diag0():
                    kt = 4 * s

                    def go():
                        ps = ps_sc.tile([128, 1024], F32, name="psc")
                        for i, r0 in enumerate((0, 64)):
                            nc.tensor.matmul(
                                ps[:, i * 512 : (i + 1) * 512],
                                lhsT=KT[
                                    r0 : r0 + 64,
                                    m * T + kt * 128 : m * T + (kt + 1) * 128,
                                ],
                                rhs=QT[r0 : r0 + 64, q0 : q0 + 512],
                                start=True,
                                stop=True,
                            )
                        nc.scalar.activation(
                            out=pt[:, aoff[kt] : aoff[kt] + 1024],
                            in_=ps[:],
                            func=AF.Exp,
                            scale=float(SCALE),
                        )
                        mask_diag(pt, aoff[kt])
                        mask_diag(pt, boff[kt])

                    return go

                def diag1():
                    kt = 4 * s + 1

                    def go():
                        ps = ps_sc.tile([128, 1024], F32, name="psc")
                        for i, r0 in enumerate((0, 64)):
                            nc.tensor.matmul(
                                ps[:, i * 512 : i * 512 + 384],
                                lhsT=KT[
                                    r0 : r0 + 64,
                                    m * T + kt * 128 : m * T + (kt + 1) * 128,
                                ],
                                rhs=QT[r0 : r0 + 64, q0 + 128 : q0 + 512],
                                start=True,
                                stop=True,
                            )
                        nc.scalar.activation(
                            out=pt[:, aoff[kt] : aoff[kt] + 384],
                            in_=ps[:, 0:384],
                            func=AF.Exp,
                            scale=float(SCALE),
                        )
                        nc.scalar.activation(
                            out=pt[:, boff[kt] : boff[kt] + 384],
                            in_=ps[:, 512:896],
                            func=AF.Exp,
                            scale=float(SCALE),
                        )
                        mask_diag(pt, aoff[kt])
                        mask_diag(pt, boff[kt])

                    return go

                def diag23():
                    kt2, kt3 = 4 * s + 2, 4 * s + 3

                    def go():
                        ps = ps_sc.tile([128, 1024], F32, name="psc")
                        # A tiles -> bank 0 (cols 0:384), B tiles -> bank 1
                        # (cols 512:896) so concurrent pairs never share a
                        # bank.
                        for i, r0 in enumerate((0, 64)):
                            nc.tensor.matmul(
                                ps[:, i * 512 : i * 512 + 256],
                                lhsT=KT[
                                    r0 : r0 + 64,
                                    m * T + kt2 * 128 : m * T + (kt2 + 1) * 128,
                                ],
                                rhs=QT[r0 : r0 + 64, q0 + 256 : q0 + 512],
                                start=True,
                                stop=True,
                            )
                        for i, r0 in enumerate((0, 64)):
                            nc.tensor.matmul(
                                ps[:, i * 512 + 256 : i * 512 + 384],
                                lhsT=KT[
                                    r0 : r0 + 64,
                                    m * T + kt3 * 128 : m * T + (kt3 + 1) * 128,
                                ],
                                rhs=QT[r0 : r0 + 64, q0 + 384 : q0 + 512],
                                start=True,
                                stop=True,
                            )
                        nc.scalar.activation(
                            out=pt[:, aoff[kt2] : aoff[kt2] + 384],
                            in_=ps[:, 0:384],
                            func=AF.Exp,
                            scale=float(SCALE),
                        )
                        nc.scalar.activation(
                            out=pt[:, boff[kt2] : boff[kt2] + 384],
                            in_=ps[:, 512:896],
                            func=AF.Exp,
                            scale=float(SCALE),
                        )
                        mask_diag(pt, aoff[kt2])
                        mask_diag(pt, boff[kt2])
                        mask_diag(pt, aoff[kt3])
                        mask_diag(pt, boff[kt3])

                    return go

                return (
                    [off_diag(kt) for kt in range(4 * s)]
                    + [diag0(), diag1(), diag23()]
                )

            def av_norm_ops(s, m, pt, split_last=False):
                """AV accumulation + transpose-free normalization.

                Returns ops: for each head i (A=even at attnT parts 0:64,
                B=odd at 64:128): AV matmul chain over kts, then
                reciprocal of the denominator row, partition_broadcast,
                and the normalizing multiply into attnT.
                """
                aoff, boff, wid = pt_layout(s)
                nk = 4 * (s + 1)
                stg = {}

                def av_group(i, kts):
                    h = 2 * m + i
                    off_of = aoff if i == 0 else boff

                    def go():
                        if i == 0 and kts[0] == 0:
                            stg["avb"] = ps_av.tile(
                                [128, 1024], F32, name="psav", tag="psav"
                            )
                        avb = stg["avb"]
                        o0 = i * 512
                        for kt in kts:
                            off = max(0, 128 * (kt - 4 * s))
                            w = 512 - off
                            nc.tensor.matmul(
                                avb[:, o0 + off : o0 + 512],
                                lhsT=vb4[:, kt * NH + h, :],
                                rhs=pt[:, off_of[kt] : off_of[kt] + w],
                                start=(kt == 0),
                                stop=(kt == nk - 1),
                            )

                    return go

                def norm(i):
                    def go():
                        avb = stg["avb"]
                        o0 = i * 512
                        rcb = rcb_pool.tile([64, 512], F32, name="rcb")
                        nc.vector.reciprocal_approx_fast(
                            out=rcb[:], in_=avb[0:64, o0 : o0 + 512]
                        )
                        nc.vector.tensor_tensor(
                            out=attnT[
                                i * 64 : (i + 1) * 64,
                                m * T + s * 512 : m * T + (s + 1) * 512,
                            ],
                            in0=avb[64:128, o0 : o0 + 512],
                            in1=rcb[:],
                            op=mybir.AluOpType.mult,
                        )

                    return go

                def groups(i):
                    ks = list(range(nk))
                    return [av_group(i, ks[j : j + 4]) for j in range(0, nk, 4)]

                return groups(0) + [norm(0)] + groups(1) + [norm(1)]

            def epi_mm(tt, i, ps, start, stop):
                lhsT = attnT[:, i * T + tt * 128 : i * T + (tt + 1) * 128]
                for ec in range(2):
                    nc.tensor.matmul(
                        ps[:, ec * 512 : (ec + 1) * 512],
                        lhsT=lhsT,
                        rhs=wo_bf[:, i * D + ec * 512 : i * D + (ec + 1) * 512],
                        start=start,
                        stop=stop,
                    )

            def epi_finish(tt, ps):
                ev = ev_pool.tile([128, 1024], BF16, name="ev")
                nc.vector.tensor_copy(ev[:], ps[:])
                eng = nc.sync if tt % 2 == 0 else nc.scalar
                eng.dma_start(out=out[tt * 128 : (tt + 1) * 128, :], in_=ev[:])

            def epi_ops(s):
                """Out-projection for slab s (4 t-tiles)."""
                ops = []
                for tt in range(4 * s, 4 * (s + 1)):
                    def go(tt=tt):
                        ps = ps_sc.tile([128, 1024], F32, name="psc")
                        epi_mm(tt, 0, ps, True, False)
                        epi_mm(tt, 1, ps, False, True)
                        epi_finish(tt, ps)

                    ops.append(go)
                return ops

            tail_ps = {}

            def tail_e0(tt):
                def go():
                    ps = ps_sc.tile([128, 1024], F32, name="psc")
                    tail_ps[tt] = ps
                    epi_mm(tt, 0, ps, True, False)

                return go

            def tail_e1(tt):
                def go():
                    ps = tail_ps[tt]
                    epi_mm(tt, 1, ps, False, True)
                    epi_finish(tt, ps)

                return go

            def interleave(a, b):
                if not a:
                    return list(b)
                if not b:
                    return list(a)
                res = []
                nb, na, bi = len(b), len(a), 0
                for i, op in enumerate(a):
                    res.append(op)
                    want = (i + 1) * nb // na
                    while bi < want:
                        res.append(b[bi])
                        bi += 1
                res.extend(b[bi:])
                return res

            # ---- pipeline ----
            duos = [(s, m) for s in range(NSLAB) for m in range(2)]
            pts = {}

            def new_pt(idx):
                pts[idx] = pt_pool.tile([128, PT_MAX], BF16, name="pt")
                return pts[idx]

            # prologue: projections for slab-0 scores of both duos; V
            # deferred past the first scores so exp starts early
            qk_chunk(wq_bf, QT, 0, 0)()
            qk_chunk(wk_bf, KT, 0, 0)()
            for op in scores_ops(0, 0, new_pt(0)):
                op()
            qk_chunk(wq_bf, QT, 1, 0)()
            qk_chunk(wk_bf, KT, 1, 0)()
            for op in scores_ops(0, 1, new_pt(1)):
                op()
            v_quad(0)()

            fillers = {
                0: [v_quad(1)],
                1: [qk_chunk(wq_bf, QT, 0, 1), qk_chunk(wk_bf, KT, 0, 1)],
                2: [qk_chunk(wq_bf, QT, 1, 1), qk_chunk(wk_bf, KT, 1, 1)],
                3: [v_quad(2)],
                4: [v_quad(3)],
            }

            for idx in range(8):
                sc = []
                if idx + 2 < 8:
                    s2, m2 = duos[idx + 2]
                    sc = scores_ops(s2, m2, new_pt(idx + 2))
                s_, m_ = duos[idx]
                avn = av_norm_ops(s_, m_, pts[idx])
                fill = fillers.get(idx, [])
                # epilogue for slab sl spread over iters 2sl+2 and 2sl+3
                epi = []
                if idx >= 2:
                    sl, half = (idx - 2) // 2, (idx - 2) % 2
                    if sl <= 2:
                        epi = epi_ops(sl)[2 * half : 2 * half + 2]
                if idx == 7:
                    # tail: pre-run the i=0 out-proj matmuls (need only
                    # duo (3,0)'s attnT) between the last AV chains
                    seq = (
                        epi
                        + avn[:-1]
                        + [tail_e0(12), tail_e0(13)]
                        + [avn[-1]]
                        + [
                            tail_e1(12),
                            tail_e0(14),
                            tail_e1(13),
                            tail_e0(15),
                            tail_e1(14),
                            tail_e1(15),
                        ]
                    )
                    for op in seq:
                        op()
                else:
                    for op in interleave(sc, fill + avn + epi):
                        op()

    nc.compile()
    return nc


def _get_nc():
    global _NC_CACHE
    if _NC_CACHE is None:
        _NC_CACHE = build()
    return _NC_CACHE


def make_in_maps(x, Wq, Wk, Wv, Wo):
    bf = ml_dtypes.bfloat16
    x = np.asarray(x, dtype=np.float32)
    Wq_ = np.asarray(Wq, dtype=np.float32)
    Wk_ = np.asarray(Wk, dtype=np.float32)
    Wv_ = np.asarray(Wv, dtype=np.float32)
    Wo_ = np.asarray(Wo, dtype=np.float32)

    def blk(a, p=128):
        # [n*p, c] -> [p, n*c] with n-blocks along columns
        n = a.shape[0] // p
        return np.ascontiguousarray(
            a.reshape(n, p, a.shape[1]).transpose(1, 0, 2).reshape(p, -1)
        )

    xTb = [blk(np.ascontiguousarray(x[b].T)).astype(bf) for b in range(2)]
    in_maps = []
    for core in range(8):
        b, g = core // 4, core % 4
        sl = slice(g * DH, (g + 1) * DH)
        in_maps.append(
            {
                "xT": xTb[b],
                "Wq": blk(Wq_[:, sl]).astype(bf),
                "Wk": blk(Wk_[:, sl]).astype(bf),
                "Wv": blk(Wv_[:, sl]).astype(bf),
                "Wo": blk(Wo_[sl, :]).astype(bf),
            }
        )
    return in_maps


def unshard(results):
    out = np.empty((2, T, D), np.float32)
    for b in range(2):
        out[b] = results[4 * b]["out"].astype(np.float32)
        for g in range(1, 4):
            out[b] += results[4 * b + g]["out"].astype(np.float32)
    return out


def kernel(x, Wq, Wk, Wv, Wo):
    nc = _get_nc()
    in_maps = make_in_maps(x, Wq, Wk, Wv, Wo)
    res = run_bass_kernel_spmd(nc, in_maps, core_ids=list(range(8)))
    return unshard(res.results)


# revision 31
# speedup vs baseline: 1.3347x; 1.0009x over previous
"""Distributed causal multi-head attention for 8 TRN2 NeuronCores.

Problem: B=2, T=2048, D=1024, H=16 heads (hd=64), f32 in/out.

Sharding: core i handles batch b=i//4 and head-group g=i%4 (4 heads).
Wq/Wk/Wv column-sharded ([1024, 256] per core), Wo row-sharded
([256, 1024] per core).  Each core computes a partial output projection
for its 4 heads over the full sequence; the host sums the 4 partials
per batch (the unshard step replaces the all-reduce).  The host
pre-casts to bf16 and pre-blocks weights/activations into the on-chip
layout so every input tensor loads with a single large DMA descriptor.

Per-core dataflow (bf16 matmuls, f32 accumulation):
  QT,KT [128(2 heads x hd),T]  per m-group; heads 2m / 2m+1 at
  partitions 0:64 / 64:128.
  Scores for a head DUO (2m, 2m+1) run as row-tiled matmul pairs
  (K=64, tile_position rows 0/64) that execute CONCURRENTLY on the
  PE array, writing separate PSUM banks; one exp (ACT) covers both.
  V is augmented with a ones column, so AV [65, 512] carries the
  softmax denominator in row 64.  Normalization is transpose-free:
  reciprocal of the denominator row (DVE), gpsimd partition_broadcast,
  then one elementwise multiply writing attnT [d, q] directly (the
  DVE applies a partition-base shift for the odd head).
  Out-projection consumes attnT per t-tile; output is written bf16
  and summed in f32 on the host.

Pipeline: scores of duo p+2 are emitted during iteration p (pt pool
is triple-buffered), giving the ACT engine a two-iteration window per
duo so exp never gates the AV stream.  QKV projection work is woven
in as PE filler ordered by first use.
"""

import numpy as np
import ml_dtypes

import concourse.bass as bass
import concourse.mybir as mybir
import concourse.tile as tile
from concourse import bacc
from concourse.bass_utils import run_bass_kernel_spmd

F32 = mybir.dt.float32
BF16 = mybir.dt.bfloat16
AF = mybir.ActivationFunctionType

T = 2048  # sequence length
D = 1024  # embed dim
NH = 4  # heads per core
HD = 64  # head dim
DH = NH * HD  # 256, sharded d per core
TT = T // 128  # 16 t tiles
DT = D // 128  # 8 embed tiles
NSLAB = 4  # q slabs of 512
SCALE = 1.0 / np.sqrt(HD)

PT_MAX = 12 * 1024 + 2560  # pt cols for s=3

_NC_CACHE = None


def pt_layout(s):
    """Per-duo pt layout: (aoff, boff, width) per kt, and total cols.

    Off-diag kt < 4s: block of 1024 at 1024*kt (A at +0, B at +512).
    Diagonal j=kt-4s: widths 512-128j packed after, A then B.
    """
    aoff, boff, wid = {}, {}, {}
    for kt in range(4 * s):
        aoff[kt], boff[kt], wid[kt] = 1024 * kt, 1024 * kt + 512, 512
    base = 4096 * s
    wid[4 * s], wid[4 * s + 1], wid[4 * s + 2], wid[4 * s + 3] = 512, 384, 256, 128
    aoff[4 * s], boff[4 * s] = base, base + 512
    aoff[4 * s + 1], boff[4 * s + 1] = base + 1024, base + 1408
    # j2+j3 packed A-then-B so the concurrent A/B matmuls hit
    # different PSUM banks: A j2|j3 contiguous, then B j2|j3
    aoff[4 * s + 2], boff[4 * s + 2] = base + 1792, base + 2176
    aoff[4 * s + 3], boff[4 * s + 3] = base + 2048, base + 2432
    return aoff, boff, wid


def build():
    nc = bacc.Bacc(None, target_bir_lowering=False, debug=False)

    xT_ext = nc.declare_dram_parameter("xT", [128, DT * T], BF16, isOutput=False)
    wq = nc.declare_dram_parameter("Wq", [128, DT * DH], BF16, isOutput=False)
    wk = nc.declare_dram_parameter("Wk", [128, DT * DH], BF16, isOutput=False)
    wv = nc.declare_dram_parameter("Wv", [128, DT * DH], BF16, isOutput=False)
    wo = nc.declare_dram_parameter("Wo", [128, 2 * D], BF16, isOutput=False)
    out = nc.declare_dram_parameter("out", [T, D], BF16, isOutput=True)

    with tile.TileContext(nc) as tc:
        with (
            tc.tile_pool(name="persist", bufs=1) as persist,
            tc.tile_pool(name="pt", bufs=3) as pt_pool,
            tc.tile_pool(name="rcb", bufs=4) as rcb_pool,
            tc.tile_pool(name="ev", bufs=2) as ev_pool,
            tc.tile_pool(name="ps_sc", bufs=2, space="PSUM") as ps_sc,
            tc.tile_pool(name="ps_av", bufs=2, space="PSUM") as ps_av,
        ):
            def P(shape, dtype, name):
                return persist.tile(shape, dtype, name=name, tag=name)

            wq_bf = P([128, DT * DH], BF16, "wq_bf")
            wk_bf = P([128, DT * DH], BF16, "wk_bf")
            wv_bf = P([128, DT * DH], BF16, "wv_bf")
            wo_bf = P([128, 2 * D], BF16, "wo_bf")
            xT = P([128, DT * T], BF16, "xT")
            QT = P([128, 2 * T], BF16, "QT")
            KT = P([128, 2 * T], BF16, "KT")
            vbuf = P([128, TT * NH * 128], BF16, "vbuf")
            attnT = P([128, 2 * T], BF16, "attnT")

            # ---- input DMAs: one descriptor per tensor (plus an early
            # xT slice for the first projection chunk) ----
            xT3 = xT.rearrange("p (dt t) -> p dt t", t=T)
            xT3e = xT_ext.rearrange("p (dt t) -> p dt t", t=T)
            # spread input DMAs across idle engines; split the two
            # startup-critical tensors so the first matmuls gate on less
            wq3, wq3e = wq_bf.rearrange("p (dt c) -> p dt c", c=DH), wq.rearrange(
                "p (dt c) -> p dt c", c=DH
            )
            nc.scalar.dma_start(out=wq3[:, 0:4, :], in_=wq3e[:, 0:4, :])
            nc.scalar.dma_start(out=wq3[:, 4:8, :], in_=wq3e[:, 4:8, :])
            nc.sync.dma_start(out=xT3[:, 0:4, 0:512], in_=xT3e[:, 0:4, 0:512])
            nc.sync.dma_start(out=xT3[:, 4:8, 0:512], in_=xT3e[:, 4:8, 0:512])
            nc.scalar.dma_start(out=wk_bf[:], in_=wk[:])
            nc.sync.dma_start(out=xT3[:, :, 512:1024], in_=xT3e[:, :, 512:1024])
            nc.gpsimd.dma_start(out=wv_bf[:], in_=wv[:])
            nc.sync.dma_start(out=xT3[:, :, 1024:T], in_=xT3e[:, :, 1024:T])
            nc.gpsimd.dma_start(out=wo_bf[:], in_=wo[:])

            # cols 0:64 of every V block are ones: the AV matmul then
            # emits the softmax denominator replicated across PSUM rows
            # 0:64 (M is free - matmul time is N-bound), giving a
            # base-0 [64, 512] operand for the fast approx-reciprocal
            # (the custom DVE op only works at partition base 0).
            vb4 = vbuf.rearrange("p (n c) -> p n c", c=128)
            nc.gpsimd.memset(vb4[:, :, 0:64], 1.0)

            # ---- emitters ----
            def qk_chunk(w_bf, outT, m, c2):
                """One QK projection chunk: 1024 t-cols of head-duo m."""
                def go():
                    ps = ps_sc.tile([128, 1024], F32, name="psc")
                    # half-outer so the first 8 matmuls only need the first
                    # 512-col xT wave (startup-critical)
                    for half in range(2):
                        c0 = c2 * 1024 + half * 512
                        for dt in range(DT):
                            nc.tensor.matmul(
                                ps[:, half * 512 : (half + 1) * 512],
                                lhsT=w_bf[
                                    :, dt * DH + m * 128 : dt * DH + (m + 1) * 128
                                ],
                                rhs=xT[:, dt * T + c0 : dt * T + c0 + 512],
                                start=(dt == 0),
                                stop=(dt == DT - 1),
                            )
                    nc.vector.tensor_copy(
                        outT[:, m * T + c2 * 1024 : m * T + (c2 + 1) * 1024], ps[:]
                    )

                return go

            def v_quad(q):
                """V projection for t-tiles 4q..4q+3."""
                def go():
                    ps = ps_sc.tile([128, 1024], F32, name="psc")
                    for ti in range(4):
                        tt = 4 * q + ti
                        for dt in range(DT):
                            nc.tensor.matmul(
                                ps[:, ti * 256 : (ti + 1) * 256],
                                lhsT=xT[:, dt * T + tt * 128 : dt * T + (tt + 1) * 128],
                                rhs=wv_bf[:, dt * DH : (dt + 1) * DH],
                                start=(dt == 0),
                                stop=(dt == DT - 1),
                            )
                    nc.vector.tensor_copy(
                        vb4[:, 4 * q * NH : 4 * (q + 1) * NH, 64:128],
                        ps.rearrange("p (t c) -> p t c", c=64),
                    )

                return go

            def mask_diag(pt, col):
                """Causal triangle on the leading 128 cols of a diag block."""
                nc.gpsimd.affine_select(
                    out=pt[:, col : col + 128],
                    in_=pt[:, col : col + 128],
                    pattern=[[1, 128]],
                    compare_op=mybir.AluOpType.is_ge,
                    fill=0.0,
                    base=0,
                    channel_multiplier=-1,
                )

            def scores_ops(s, m, pt):
                """Row-tiled concurrent score pairs + exp for duo (s, m)."""
                aoff, boff, wid = pt_layout(s)
                q0 = m * T + s * 512

                def off_diag(kt):
                    def go():
                        ps = ps_sc.tile([128, 1024], F32, name="psc")
                        for i, r0 in enumerate((0, 64)):
                            nc.tensor.matmul(
                                ps[:, i * 512 : (i + 1) * 512],
                                lhsT=KT[
                                    r0 : r0 + 64,
                                    m * T + kt * 128 : m * T + (kt + 1) * 128,
                                ],
                                rhs=QT[r0 : r0 + 64, q0 : q0 + 512],
                                start=True,
                                stop=True,
                            )
                        nc.scalar.activation(
                            out=pt[:, aoff[kt] : aoff[kt] + 1024],
                            in_=ps[:],
                            func=AF.Exp,
                            scale=float(SCALE),
                        )

                    return go

                def diag0():
                    kt = 4 * s

                    def go():
                        ps = ps_sc.tile([128, 1024], F32, name="psc")
                        for i, r0 in enumerate((0, 64)):
                            nc.tensor.matmul(
                                ps[:, i * 512 : (i + 1) * 512],
                                lhsT=KT[
                                    r0 : r0 + 64,
                                    m * T + kt * 128 : m * T + (kt + 1) * 128,
                                ],
                                rhs=QT[r0 : r0 + 64, q0 : q0 + 512],
                                start=True,
                                stop=True,
                            )
                        nc.scalar.activation(
                            out=pt[:, aoff[kt] : aoff[kt] + 1024],
                            in_=ps[:],
                            func=AF.Exp,
                            scale=float(SCALE),
                        )
                        mask_diag(pt, aoff[kt])
                        mask_diag(pt, boff[kt])

                    return go

                def diag1():
                    kt = 4 * s + 1

                    def go():
                        ps = ps_sc.tile([128, 1024], F32, name="psc")
                        for i, r0 in enumerate((0, 64)):
                            nc.tensor.matmul(
                                ps[:, i * 512 : i * 512 + 384],
                                lhsT=KT[
                                    r0 : r0 + 64,
                                    m * T + kt * 128 : m * T + (kt + 1) * 128,
                                ],
                                rhs=QT[r0 : r0 + 64, q0 + 128 : q0 + 512],
                                start=True,
                                stop=True,
                            )
                        nc.scalar.activation(
                            out=pt[:, aoff[kt] : aoff[kt] + 384],
                            in_=ps[:, 0:384],
                            func=AF.Exp,
                            scale=float(SCALE),
                        )
                        nc.scalar.activation(
                            out=pt[:, boff[kt] : boff[kt] + 384],
                            in_=ps[:, 512:896],
                            func=AF.Exp,
                            scale=float(SCALE),
                        )
                        mask_diag(pt, aoff[kt])
                        mask_diag(pt, boff[kt])

                    return go

                def diag23():
                    kt2, kt3 = 4 * s + 2, 4 * s + 3

                    def go():
                        ps = ps_sc.tile([128, 1024], F32, name="psc")
                        # A tiles -> bank 0 (cols 0:384), B tiles -> bank 1
                        # (cols 512:896) so concurrent pairs never share a
                        # bank.
                        for i, r0 in enumerate((0, 64)):
                            nc.tensor.matmul(
                                ps[:, i * 512 : i * 512 + 256],
                                lhsT=KT[
                                    r0 : r0 + 64,
                                    m * T + kt2 * 128 : m * T + (kt2 + 1) * 128,
                                ],
                                rhs=QT[r0 : r0 + 64, q0 + 256 : q0 + 512],
                                start=True,
                                stop=True,
                            )
                        for i, r0 in enumerate((0, 64)):
                            nc.tensor.matmul(
                                ps[:, i * 512 + 256 : i * 512 + 384],
                                lhsT=KT[
                                    r0 : r0 + 64,
                                    m * T + kt3 * 128 : m * T + (kt3 + 1) * 128,
                                ],
                                rhs=QT[r0 : r0 + 64, q0 + 384 : q0 + 512],
                                start=True,
                                stop=True,
                            )
                        nc.scalar.activation(
                            out=pt[:, aoff[kt2] : aoff[kt2] + 384],
                            in_=ps[:, 0:384],
                            func=AF.Exp,
                            scale=float(SCALE),
                        )
                        nc.scalar.activation(
                            out=pt[:, boff[kt2] : boff[kt2] + 384],
                            in_=ps[:, 512:896],
                            func=AF.Exp,
                            scale=float(SCALE),
                        )
                        mask_diag(pt, aoff[kt2])
                        mask_diag(pt, boff[kt2])
                        mask_diag(pt, aoff[kt3])
                        mask_diag(pt, boff[kt3])

                    return go

                return (
                    [off_diag(kt) for kt in range(4 * s)]
                    + [diag0(), diag1(), diag23()]
                )

            def av_norm_ops(s, m, pt, split_last=False):
                """AV accumulation + transpose-free normalization.

                Returns ops: for each head i (A=even at attnT parts 0:64,
                B=odd at 64:128): AV matmul chain over kts, then
                reciprocal of the denominator row, partition_broadcast,
                and the normalizing multiply into attnT.
                """
                aoff, boff, wid = pt_layout(s)
                nk = 4 * (s + 1)
                stg = {}

                def av_group(i, kts):
                    h = 2 * m + i
                    off_of = aoff if i == 0 else boff

                    def go():
                        if i == 0 and kts[0] == 0:
                            stg["avb"] = ps_av.tile(
                                [128, 1024], F32, name="psav", tag="psav"
                            )
                        avb = stg["avb"]
                        o0 = i * 512
                        for kt in kts:
                            off = max(0, 128 * (kt - 4 * s))
                            w = 512 - off
                            nc.tensor.matmul(
                                avb[:, o0 + off : o0 + 512],
                                lhsT=vb4[:, kt * NH + h, :],
                                rhs=pt[:, off_of[kt] : off_of[kt] + w],
                                start=(kt == 0),
                                stop=(kt == nk - 1),
                            )

                    return go

                def norm(i):
                    def go():
                        avb = stg["avb"]
                        o0 = i * 512
                        rcb = rcb_pool.tile([64, 512], F32, name="rcb")
                        nc.vector.reciprocal_approx_fast(
                            out=rcb[:], in_=avb[0:64, o0 : o0 + 512]
                        )
                        nc.vector.tensor_tensor(
                            out=attnT[
                                i * 64 : (i + 1) * 64,
                                m * T + s * 512 : m * T + (s + 1) * 512,
                            ],
                            in0=avb[64:128, o0 : o0 + 512],
                            in1=rcb[:],
                            op=mybir.AluOpType.mult,
                        )

                    return go

                def groups(i):
                    ks = list(range(nk))
                    gs = [av_group(i, ks[j : j + 4]) for j in range(0, nk, 4)]
                    # norm rides on the last AV group: norms are DVE-only
                    # and must not count as PE padding in the interleave
                    last, nrm = gs[-1], norm(i)

                    def last_plus_norm():
                        last()
                        nrm()

                    gs[-1] = last_plus_norm
                    return gs

                return groups(0) + groups(1)

            def epi_mm(tt, i, ps, start, stop):
                lhsT = attnT[:, i * T + tt * 128 : i * T + (tt + 1) * 128]
                for ec in range(2):
                    nc.tensor.matmul(
                        ps[:, ec * 512 : (ec + 1) * 512],
                        lhsT=lhsT,
                        rhs=wo_bf[:, i * D + ec * 512 : i * D + (ec + 1) * 512],
                        start=start,
                        stop=stop,
                    )

            def epi_finish(tt, ps, tail=False):
                ev = ev_pool.tile([128, 1024], BF16, name="ev")
                if not tail:
                    nc.vector.tensor_copy(ev[:], ps[:])
                    eng = nc.sync if tt % 2 == 0 else nc.scalar
                    eng.dma_start(
                        out=out[tt * 128 : (tt + 1) * 128, :], in_=ev[:]
                    )
                    return
                # tail: halves cast in parallel on DVE + ACT, two DMAs
                nc.vector.tensor_copy(ev[:, 0:512], ps[:, 0:512])
                nc.scalar.copy(ev[:, 512:1024], ps[:, 512:1024])
                nc.sync.dma_start(
                    out=out[tt * 128 : (tt + 1) * 128, 0:512], in_=ev[:, 0:512]
                )
                nc.gpsimd.dma_start(
                    out=out[tt * 128 : (tt + 1) * 128, 512:1024],
                    in_=ev[:, 512:1024],
                )

            def epi_ops(s):
                """Out-projection for slab s (4 t-tiles)."""
                ops = []
                for tt in range(4 * s, 4 * (s + 1)):
                    def go(tt=tt):
                        ps = ps_sc.tile([128, 1024], F32, name="psc")
                        epi_mm(tt, 0, ps, True, False)
                        epi_mm(tt, 1, ps, False, True)
                        epi_finish(tt, ps)

                    ops.append(go)
                return ops

            tail_ps = {}

            def tail_e0(tt):
                def go():
                    ps = ps_sc.tile([128, 1024], F32, name="psc")
                    tail_ps[tt] = ps
                    epi_mm(tt, 0, ps, True, False)

                return go

            def tail_e1(tt):
                def go():
                    ps = tail_ps[tt]
                    epi_mm(tt, 1, ps, False, True)
                    epi_finish(tt, ps, tail=True)

                return go

            def interleave(a, b):
                if not a:
                    return list(b)
                if not b:
                    return list(a)
                res = []
                nb, na, bi = len(b), len(a), 0
                for i, op in enumerate(a):
                    res.append(op)
                    want = (i + 1) * nb // na
                    while bi < want:
                        res.append(b[bi])
                        bi += 1
                res.extend(b[bi:])
                return res

            # ---- pipeline ----
            duos = [(s, m) for s in range(NSLAB) for m in range(2)]
            pts = {}

            def new_pt(idx):
                pts[idx] = pt_pool.tile([128, PT_MAX], BF16, name="pt")
                return pts[idx]

            # prologue: projections for slab-0 scores of both duos; V
            # deferred past the first scores so exp starts early
            qk_chunk(wq_bf, QT, 0, 0)()
            qk_chunk(wk_bf, KT, 0, 0)()
            for op in scores_ops(0, 0, new_pt(0)):
                op()
            qk_chunk(wq_bf, QT, 1, 0)()
            qk_chunk(wk_bf, KT, 1, 0)()
            for op in scores_ops(0, 1, new_pt(1)):
                op()
            v_quad(0)()

            fillers = {
                0: [v_quad(1)],
                1: [qk_chunk(wq_bf, QT, 0, 1), qk_chunk(wk_bf, KT, 0, 1)],
                2: [qk_chunk(wq_bf, QT, 1, 1), qk_chunk(wk_bf, KT, 1, 1)],
                3: [v_quad(2)],
                4: [v_quad(3)],
            }

            for idx in range(8):
                sc = []
                if idx + 2 < 8:
                    s2, m2 = duos[idx + 2]
                    sc = scores_ops(s2, m2, new_pt(idx + 2))
                s_, m_ = duos[idx]
                avn = av_norm_ops(s_, m_, pts[idx])
                fill = fillers.get(idx, [])
                # epilogue for slab sl spread over iters 2sl+2 and 2sl+3
                epi = []
                if idx >= 2:
                    sl, half = (idx - 2) // 2, (idx - 2) % 2
                    if sl <= 2:
                        epi = epi_ops(sl)[2 * half : 2 * half + 2]
                if idx == 7:
                    # tail: pre-run the i=0 out-proj matmuls (need only
                    # duo (3,0)'s attnT) while the last norm chain drains
                    seq = (
                        epi
                        + avn
                        + [
                            tail_e0(12),
                            tail_e0(13),
                            tail_e1(12),
                            tail_e0(14),
                            tail_e1(13),
                            tail_e0(15),
                            tail_e1(14),
                            tail_e1(15),
                        ]
                    )
                    for op in seq:
                        op()
                else:
                    for op in interleave(sc, fill + avn + epi):
                        op()

    nc.compile()
    return nc


def _get_nc():
    global _NC_CACHE
    if _NC_CACHE is None:
        _NC_CACHE = build()
    return _NC_CACHE


def make_in_maps(x, Wq, Wk, Wv, Wo):
    bf = ml_dtypes.bfloat16
    x = np.asarray(x, dtype=np.float32)
    Wq_ = np.asarray(Wq, dtype=np.float32)
    Wk_ = np.asarray(Wk, dtype=np.float32)
    Wv_ = np.asarray(Wv, dtype=np.float32)
    Wo_ = np.asarray(Wo, dtype=np.float32)

    def blk(a, p=128):
        # [n*p, c] -> [p, n*c] with n-blocks along columns
        n = a.shape[0] // p
        return np.ascontiguousarray(
            a.reshape(n, p, a.shape[1]).transpose(1, 0, 2).reshape(p, -1)
        )

    xTb = [blk(np.ascontiguousarray(x[b].T)).astype(bf) for b in range(2)]
    in_maps = []
    for core in range(8):
        b, g = core // 4, core % 4
        sl = slice(g * DH, (g + 1) * DH)
        in_maps.append(
            {
                "xT": xTb[b],
                "Wq": blk(Wq_[:, sl]).astype(bf),
                "Wk": blk(Wk_[:, sl]).astype(bf),
                "Wv": blk(Wv_[:, sl]).astype(bf),
                "Wo": blk(Wo_[sl, :]).astype(bf),
            }
        )
    return in_maps


def unshard(results):
    out = np.empty((2, T, D), np.float32)
    for b in range(2):
        out[b] = results[4 * b]["out"].astype(np.float32)
        for g in range(1, 4):
            out[b] += results[4 * b + g]["out"].astype(np.float32)
    return out


def kernel(x, Wq, Wk, Wv, Wo):
    nc = _get_nc()
    in_maps = make_in_maps(x, Wq, Wk, Wv, Wo)
    res = run_bass_kernel_spmd(nc, in_maps, core_ids=list(range(8)))
    return unshard(res.results)
